# revision 46
# baseline (speedup 1.0000x reference)
"""Trainium2 Bass kernel for nn_Meta_Graph1_40114994545303 (gnn_message_passing).

Math: the reference returns only the global-node row of the GCN output.
With mask = (attribute_label > 0), star adjacency means
    out[s, :] = tanh( (sum_a mask[s,a] * attribute_feat[s,a,:]) @ W + b )
and x never reaches the output (adj[A, A] = 0). Data-parallel over batch,
32 samples per core on 8 cores; the kernel is HBM-bandwidth-bound, so:

- Dead-input elimination: rows with mask 0 have structurally-zero
  coefficients in the adjacency operand (same as x, which is never shipped),
  so the host stages only the live feat rows (~2.3MB vs 4MB), zero-padded to
  a 128-row-chunk multiple, partition-major so the DMA moves 16KB-contiguous
  runs per partition at line rate.
- Stage 1: masked sum as block-diagonal matmul (mask stationary, feat
  moving, four 512-col tiles packed in one PSUM bank via tile_position);
  DVE 32x32 block transposes (batched 4 blocks/instruction) build the
  stage-2 stationary, hidden under the W stream.
- Stage 2 chases the W stream k2-major; W replicated per core (collectives
  measure ~90us for even a 128KB AllGather on this runtime -- not viable).
- Bias folded in as a rank-1 matmul accumulated first (off the tail path);
  one full-width tanh on the scalar engine, then the output DMAs split
  across both HWDGE queues.
"""

import numpy as np

import concourse.bacc as bacc
import concourse.mybir as mybir

B, A, D = 256, 32, 2048
NCORES = 8
S = B // NCORES  # 32 samples per core
P = 128
KC2 = D // P  # 16 k-chunks in stage 2 (contraction over d_in)
NT = D // 512  # 4 psum-bank-wide column tiles
F32 = mybir.dt.float32
F16 = mybir.dt.float16

# W stream split across both HWDGE queues. Stage 2 consumes chunks in k2
# order, so the LAST bytes to arrive must be the LAST chunks in PE order:
# scalar carries the early chunks 0..5 (they land while sync still streams),
# sync carries 6..15 behind feat and finishes with chunk 15 — only its 4
# matmuls remain after the stream ends.
WCH = [4, 2, 4, 4, 2]
WST = [0, 4, 6, 10, 14]
NW = len(WCH)
W_SYNC_GROUPS = (2, 3, 4)
W_SCALAR_GROUPS = (0, 1)


def build_nc(nch: int):
    cdt = F16
    nc = bacc.Bacc("TRN2", target_bir_lowering=False, debug=False)

    featd = nc.dram_tensor("feat", [P, nch * D], cdt, kind="ExternalInput")
    wd = nc.dram_tensor("w", [P, KC2 * D], cdt, kind="ExternalInput")
    mbdt = nc.dram_tensor("mbdt", [P, nch * S], cdt, kind="ExternalInput")
    bias = nc.dram_tensor("bias", [1, D], cdt, kind="ExternalInput")
    onesd = nc.dram_tensor("ones", [1, S], cdt, kind="ExternalInput")
    out = nc.dram_tensor("out", [S, D], F32, kind="ExternalOutput")

    from contextlib import ExitStack

    with ExitStack() as ctx:
        feat_sb = ctx.enter_context(nc.sbuf_tensor([P, nch, D], cdt))
        w_sb = ctx.enter_context(nc.sbuf_tensor([P, KC2, D], cdt))
        mbdt_sb = ctx.enter_context(nc.sbuf_tensor([P, nch, S], cdt))
        bias_sb = ctx.enter_context(nc.sbuf_tensor([1, D], cdt))
        ones_sb = ctx.enter_context(nc.sbuf_tensor([1, S], cdt))
        msc_sb = ctx.enter_context(nc.sbuf_tensor([P, 512], cdt))
        msT_sb = ctx.enter_context(nc.sbuf_tensor([P, KC2, S], cdt))
        out_sb = ctx.enter_context(nc.sbuf_tensor([P, 512], F32))
        pm_bank = ctx.enter_context(nc.psum_tensor([P, 512], F32))
        po_bank = ctx.enter_context(nc.psum_tensor([P, 512], F32))
        fsems = [ctx.enter_context(nc.semaphore(f"fs{g}")) for g in range(2)]
        wsems = [ctx.enter_context(nc.semaphore(f"ws{g}")) for g in range(NW)]
        csem = ctx.enter_context(nc.semaphore("csem"))
        s1_sem = ctx.enter_context(nc.semaphore("s1_sem"))
        tr_sem = ctx.enter_context(nc.semaphore("tr_sem"))
        s2_sem = ctx.enter_context(nc.semaphore("s2_sem"))
        act_sem = ctx.enter_context(nc.semaphore("act_sem"))
        osem = ctx.enter_context(nc.semaphore("osem"))
        block = ctx.enter_context(nc.Block(no_gpsimd_drain=True))

        # feat DMA split points (chunk counts per group)
        if nch == 1:
            FS = [(0, 1)]
        else:
            FS = [(0, (nch + 1) // 2), ((nch + 1) // 2, nch)]

        def w_dma(eng, g):
            st, ln = WST[g], WCH[g]
            eng.dma_start(
                w_sb[:, st : st + ln, :],
                wd[:, st * D : (st + ln) * D].rearrange("p (c d) -> p c d", d=D),
            ).then_inc(wsems[g], 16)

        @block.sync
        def _(sync):
            for g, (a0, a1) in enumerate(FS):
                sync.dma_start(
                    feat_sb[:, a0:a1, :],
                    featd[:, a0 * D : a1 * D].rearrange("p (c d) -> p c d", d=D),
                ).then_inc(fsems[g], 16)
            for g in W_SYNC_GROUPS:
                w_dma(sync, g)
            sync.wait_ge(act_sem, 1)
            for n in (0, 2):
                sync.dma_start(
                    out[:, n * 512 : (n + 1) * 512], out_sb[n * S : (n + 1) * S, :]
                ).then_inc(osem, 16)
            sync.wait_ge(osem, 32)

        @block.scalar
        def _(scalar):
            # tiny consts first on the otherwise-idle scalar queue, then the
            # tail of the W stream (arrives early, off the stage-2 pace)
            scalar.dma_start(
                mbdt_sb[:], mbdt[:].rearrange("p (k j) -> p k j", k=nch)
            ).then_inc(csem, 16)
            scalar.dma_start(bias_sb[:], bias[:]).then_inc(csem, 16)
            scalar.dma_start(ones_sb[:], onesd[:]).then_inc(csem, 16)
            for g in W_SCALAR_GROUPS:
                w_dma(scalar, g)
            scalar.wait_ge(s2_sem, NT)
            nc.scalar.activation(
                out_sb[:], po_bank[:], mybir.ActivationFunctionType.Tanh
            ).then_inc(act_sem, 1)
            scalar.wait_ge(act_sem, 1)
            for n in (1, 3):
                scalar.dma_start(
                    out[:, n * 512 : (n + 1) * 512], out_sb[n * S : (n + 1) * S, :]
                ).then_inc(osem, 16)
            scalar.wait_ge(osem, 32)

        @block.vector
        def _(vector):
            # s (stage-1 psum) -> fp16, then 32x32 block transposes into the
            # stage-2 stationary; 4 strided blocks per DVE instruction
            vector.wait_ge(s1_sem, 1)
            nc.vector.tensor_copy(msc_sb[:], pm_bank[:])
            nc.vector.drain()
            # 32x32 block transposes, 4 strided blocks per DVE instruction:
            # blocks (n, q=rg+4t) share output partition rows rg*32 and map to
            # k2 = 4n+t
            lastt = None
            for n in range(NT):
                for rg in range(NT):
                    lastt = nc.vector.transpose(
                        msT_sb[rg * S : (rg + 1) * S, 4 * n : 4 * n + 4, :],
                        msc_sb[n * S : (n + 1) * S, :]
                        .rearrange("p (c q j) -> p c q j", q=NT, j=S)[
                            :, :, rg : rg + 1, :
                        ]
                        .rearrange("p c q j -> p (c q) j"),
                    )
            lastt.then_inc(tr_sem, 1)

        @block.tensor
        def _(tensor):
            tensor.wait_ge(csem, 48)  # mbdt/bias/ones resident
            # bias as the FIRST accumulation into po_bank (off the tail path)
            for n in range(NT):
                nc.tensor.matmul(
                    po_bank[n * S : (n + 1) * S, :],
                    ones_sb[:],
                    bias_sb[:, n * 512 : (n + 1) * 512],
                    start=True,
                    stop=False,
                    tile_position=(0, n * S),
                    skip_group_check=True,
                )
            # stage 1: s[j, d] = sum_slot mbd[slot, j] * feat[slot, d]
            # (mask stationary, feat moving; 4 column tiles packed into one
            # PSUM bank at partition offsets 0/32/64/96)
            last = None
            for k in range(nch):
                for g, (a0, _) in enumerate(FS):
                    if k == a0:
                        tensor.wait_ge(fsems[g], 16)
                for n in range(NT):
                    last = nc.tensor.matmul(
                        pm_bank[n * S : (n + 1) * S, :],
                        mbdt_sb[:, k, :],
                        feat_sb[:, k, n * 512 : (n + 1) * 512],
                        start=(k == 0),
                        stop=(k == nch - 1),
                        tile_position=(0, n * S),
                        skip_group_check=True,
                    )
            last.then_inc(s1_sem, 1)
            tensor.wait_ge(tr_sem, 1)
            # stage 2 k2-major so the PE chases the W stream; at the final
            # k-chunk each column tile signals s2 so tanh/output pipeline
            for g in range(NW):
                tensor.wait_ge(wsems[g], 16)
                for c in range(WCH[g]):
                    k2 = WST[g] + c
                    for n in range(NT):
                        mm = nc.tensor.matmul(
                            po_bank[n * S : (n + 1) * S, :],
                            msT_sb[:, k2, :],
                            w_sb[:, k2, n * 512 : (n + 1) * 512],
                            start=False,
                            stop=(k2 == KC2 - 1),
                            tile_position=(0, n * S),
                            skip_group_check=True,
                        )
                        if k2 == KC2 - 1:
                            mm.then_inc(s2_sem, 1)

    nc.compile()
    return nc


def _pm(x, nchunks):
    d = x.shape[1]
    return np.ascontiguousarray(
        x.reshape(nchunks, P, d).transpose(1, 0, 2).reshape(P, nchunks * d)
    )


def _host_prep(inputs: dict):
    feat = np.asarray(inputs["attribute_feat"], dtype=np.float32)
    label = np.asarray(inputs["attribute_label"])
    w = np.asarray(inputs["W"], dtype=np.float32).astype(np.float16)
    b = np.asarray(inputs["b"], dtype=np.float32).reshape(1, D).astype(np.float16)
    mask = label > 0

    w_pm = _pm(w, KC2)
    ones = np.ones((1, S), np.float16)

    rows_per_core = [
        np.nonzero(mask[c * S : (c + 1) * S].reshape(-1))[0] for c in range(NCORES)
    ]
    nch = max(1, int(np.ceil(max(len(r) for r in rows_per_core) / P)))

    in_maps = []
    for c in range(NCORES):
        rows = rows_per_core[c]
        nslot = nch * P
        feat_c = feat[c * S : (c + 1) * S].reshape(S * A, D)
        feat_cmp = np.zeros((nslot, D), np.float16)
        feat_cmp[: len(rows)] = feat_c[rows].astype(np.float16)
        mbd = np.zeros((nch, P, S), np.float32)
        for i, r in enumerate(rows):
            mbd[i // P, i % P, r // A] = 1.0
        in_maps.append(
            {
                "feat": _pm(feat_cmp, nch),
                "mbdt": np.ascontiguousarray(mbd.transpose(1, 0, 2))
                .reshape(P, nch * S)
                .astype(np.float16),
                "w": w_pm,
                "bias": b,
                "ones": ones,
            }
        )
    return in_maps, nch


_NC_CACHE: dict = {}


def run(inputs: dict, trace: bool = False):
    from concourse.bass_utils import run_bass_kernel_spmd

    in_maps, nch = _host_prep(inputs)
    if nch not in _NC_CACHE:
        _NC_CACHE[nch] = build_nc(nch)
    nc = _NC_CACHE[nch]
    res = run_bass_kernel_spmd(nc, in_maps, list(range(NCORES)), trace=trace)
    out = np.concatenate([res.results[c]["out"] for c in range(NCORES)], axis=0)
    return out, res


def kernel(**inputs) -> np.ndarray:
    out, _ = run(inputs)
    return out


# revision 50
# speedup vs baseline: 1.0920x; 1.0920x over previous
"""Trainium2 Bass kernel for nn_Meta_Graph1_40114994545303 (gnn_message_passing).

Math: the reference returns only the global-node row of the GCN output.
With mask = (attribute_label > 0), star adjacency means
    out[s, :] = tanh( (sum_a mask[s,a] * attribute_feat[s,a,:]) @ W + b )
and x never reaches the output (adj[A, A] = 0). Data-parallel over batch,
32 samples per core on 8 cores; the kernel is HBM-bandwidth-bound, so:

- Dead-input elimination: rows with mask 0 have structurally-zero
  coefficients in the adjacency operand (same as x, which is never shipped),
  so the host stages only the live feat rows (~2.3MB vs 4MB), zero-padded to
  a 128-row-chunk multiple, partition-major so the DMA moves 16KB-contiguous
  runs per partition at line rate.
- Stage 1: masked sum as block-diagonal matmul (mask stationary, feat
  moving, four 512-col tiles packed in one PSUM bank via tile_position);
  DVE 32x32 block transposes (batched 4 blocks/instruction) build the
  stage-2 stationary, hidden under the W stream.
- Stage 2 chases the W stream k2-major; W replicated per core (collectives
  measure ~90us for even a 128KB AllGather on this runtime -- not viable).
- Bias folded in as a rank-1 matmul accumulated first (off the tail path);
  one full-width tanh on the scalar engine, then the output DMAs split
  across both HWDGE queues.
"""

import numpy as np

import concourse.bacc as bacc
import concourse.mybir as mybir

B, A, D = 256, 32, 2048
NCORES = 8
S = B // NCORES  # 32 samples per core
P = 128
KC2 = D // P  # 16 k-chunks in stage 2 (contraction over d_in)
NT = D // 512  # 4 psum-bank-wide column tiles
F32 = mybir.dt.float32
F16 = mybir.dt.float16

# W stream split across both HWDGE queues. Stage 2 consumes chunks in k2
# order, so the LAST bytes to arrive must be the LAST chunks in PE order:
# scalar carries the early chunks 0..5 (they land while sync still streams),
# sync carries 6..15 behind feat and finishes with chunk 15 — only its 4
# matmuls remain after the stream ends.
WCH = [4, 2, 4, 4, 2]
WST = [0, 4, 6, 10, 14]
NW = len(WCH)
W_SYNC_GROUPS = (2, 3, 4)
W_SCALAR_GROUPS = (0, 1)


def build_nc(nch: int):
    cdt = F16
    nc = bacc.Bacc("TRN2", target_bir_lowering=False, debug=False)

    featd = nc.dram_tensor("feat", [P, nch * D], cdt, kind="ExternalInput")
    wd = nc.dram_tensor("w", [P, KC2 * D], cdt, kind="ExternalInput")
    mbdt = nc.dram_tensor("mbdt", [P, nch * S], cdt, kind="ExternalInput")
    bias = nc.dram_tensor("bias", [1, D], cdt, kind="ExternalInput")
    onesd = nc.dram_tensor("ones", [1, S], cdt, kind="ExternalInput")
    out = nc.dram_tensor("out", [S, D], F32, kind="ExternalOutput")

    from contextlib import ExitStack

    with ExitStack() as ctx:
        feat_sb = ctx.enter_context(nc.sbuf_tensor([P, nch, D], cdt))
        w_sb = ctx.enter_context(nc.sbuf_tensor([P, KC2, D], cdt))
        mbdt_sb = ctx.enter_context(nc.sbuf_tensor([P, nch, S], cdt))
        bias_sb = ctx.enter_context(nc.sbuf_tensor([1, D], cdt))
        ones_sb = ctx.enter_context(nc.sbuf_tensor([1, S], cdt))
        msc_sb = ctx.enter_context(nc.sbuf_tensor([P, 512], cdt))
        msT_sb = ctx.enter_context(nc.sbuf_tensor([P, KC2, S], cdt))
        out_sb = ctx.enter_context(nc.sbuf_tensor([P, 512], F32))
        pm_bank = ctx.enter_context(nc.psum_tensor([P, 512], F32))
        po_bank = ctx.enter_context(nc.psum_tensor([P, 512], F32))
        fsems = [ctx.enter_context(nc.semaphore(f"fs{g}")) for g in range(2)]
        wsems = [ctx.enter_context(nc.semaphore(f"ws{g}")) for g in range(NW)]
        csem = ctx.enter_context(nc.semaphore("csem"))
        s1_sem = ctx.enter_context(nc.semaphore("s1_sem"))
        tr_sem = ctx.enter_context(nc.semaphore("tr_sem"))
        s2_sem = ctx.enter_context(nc.semaphore("s2_sem"))
        act_sem = ctx.enter_context(nc.semaphore("act_sem"))
        osem = ctx.enter_context(nc.semaphore("osem"))
        block = ctx.enter_context(nc.Block(no_gpsimd_drain=True))

        # feat DMA split points (chunk counts per group)
        if nch == 1:
            FS = [(0, 1)]
        else:
            FS = [(0, (nch + 1) // 2), ((nch + 1) // 2, nch)]

        def w_dma(eng, g):
            st, ln = WST[g], WCH[g]
            eng.dma_start(
                w_sb[:, st : st + ln, :],
                wd[:, st * D : (st + ln) * D].rearrange("p (c d) -> p c d", d=D),
            ).then_inc(wsems[g], 16)

        @block.sync
        def _(sync):
            for g, (a0, a1) in enumerate(FS):
                sync.dma_start(
                    feat_sb[:, a0:a1, :],
                    featd[:, a0 * D : a1 * D].rearrange("p (c d) -> p c d", d=D),
                ).then_inc(fsems[g], 16)
            for g in W_SYNC_GROUPS:
                w_dma(sync, g)
            sync.wait_ge(act_sem, 1)
            # single out DMA: SBUF side stays 2D [128, 512]; the DRAM side is
            # a 3D AP (n, j, c) matching the (n j) partition packing
            sync.dma_start(
                out[:].rearrange("j (n c) -> n j c", c=512),
                out_sb[:],
            ).then_inc(osem, 16)
            sync.wait_ge(osem, 16)

        @block.scalar
        def _(scalar):
            # tiny consts first on the otherwise-idle scalar queue, then the
            # tail of the W stream (arrives early, off the stage-2 pace)
            scalar.dma_start(
                mbdt_sb[:], mbdt[:].rearrange("p (k j) -> p k j", k=nch)
            ).then_inc(csem, 16)
            scalar.dma_start(bias_sb[:], bias[:]).then_inc(csem, 16)
            scalar.dma_start(ones_sb[:], onesd[:]).then_inc(csem, 16)
            for g in W_SCALAR_GROUPS:
                w_dma(scalar, g)
            scalar.wait_ge(s2_sem, NT)
            nc.scalar.activation(
                out_sb[:], po_bank[:], mybir.ActivationFunctionType.Tanh
            ).then_inc(act_sem, 1)

        @block.vector
        def _(vector):
            # s (stage-1 psum) -> fp16, then 32x32 block transposes into the
            # stage-2 stationary; 4 strided blocks per DVE instruction
            vector.wait_ge(s1_sem, 1)
            nc.vector.tensor_copy(msc_sb[:], pm_bank[:])
            nc.vector.drain()
            # 32x32 block transposes, 4 strided blocks per DVE instruction:
            # blocks (n, q=rg+4t) share output partition rows rg*32 and map to
            # k2 = 4n+t
            lastt = None
            for n in range(NT):
                for rg in range(NT):
                    lastt = nc.vector.transpose(
                        msT_sb[rg * S : (rg + 1) * S, 4 * n : 4 * n + 4, :],
                        msc_sb[n * S : (n + 1) * S, :]
                        .rearrange("p (c q j) -> p c q j", q=NT, j=S)[
                            :, :, rg : rg + 1, :
                        ]
                        .rearrange("p c q j -> p (c q) j"),
                    )
            lastt.then_inc(tr_sem, 1)

        @block.tensor
        def _(tensor):
            tensor.wait_ge(csem, 48)  # mbdt/bias/ones resident
            # bias as the FIRST accumulation into po_bank (off the tail path)
            for n in range(NT):
                nc.tensor.matmul(
                    po_bank[n * S : (n + 1) * S, :],
                    ones_sb[:],
                    bias_sb[:, n * 512 : (n + 1) * 512],
                    start=True,
                    stop=False,
                    tile_position=(0, n * S),
                    skip_group_check=True,
                )
            # stage 1: s[j, d] = sum_slot mbd[slot, j] * feat[slot, d]
            # (mask stationary, feat moving; 4 column tiles packed into one
            # PSUM bank at partition offsets 0/32/64/96)
            last = None
            for k in range(nch):
                for g, (a0, _) in enumerate(FS):
                    if k == a0:
                        tensor.wait_ge(fsems[g], 16)
                for n in range(NT):
                    last = nc.tensor.matmul(
                        pm_bank[n * S : (n + 1) * S, :],
                        mbdt_sb[:, k, :],
                        feat_sb[:, k, n * 512 : (n + 1) * 512],
                        start=(k == 0),
                        stop=(k == nch - 1),
                        tile_position=(0, n * S),
                        skip_group_check=True,
                    )
            last.then_inc(s1_sem, 1)
            tensor.wait_ge(tr_sem, 1)
            # stage 2 k2-major so the PE chases the W stream; at the final
            # k-chunk each column tile signals s2 so tanh/output pipeline
            for g in range(NW):
                tensor.wait_ge(wsems[g], 16)
                for c in range(WCH[g]):
                    k2 = WST[g] + c
                    for n in range(NT):
                        mm = nc.tensor.matmul(
                            po_bank[n * S : (n + 1) * S, :],
                            msT_sb[:, k2, :],
                            w_sb[:, k2, n * 512 : (n + 1) * 512],
                            start=False,
                            stop=(k2 == KC2 - 1),
                            tile_position=(0, n * S),
                            skip_group_check=True,
                        )
                        if k2 == KC2 - 1:
                            mm.then_inc(s2_sem, 1)

    nc.compile()
    return nc


def _pm(x, nchunks):
    d = x.shape[1]
    return np.ascontiguousarray(
        x.reshape(nchunks, P, d).transpose(1, 0, 2).reshape(P, nchunks * d)
    )


def _host_prep(inputs: dict):
    feat = np.asarray(inputs["attribute_feat"], dtype=np.float32)
    label = np.asarray(inputs["attribute_label"])
    w = np.asarray(inputs["W"], dtype=np.float32).astype(np.float16)
    b = np.asarray(inputs["b"], dtype=np.float32).reshape(1, D).astype(np.float16)
    mask = label > 0

    w_pm = _pm(w, KC2)
    ones = np.ones((1, S), np.float16)

    rows_per_core = [
        np.nonzero(mask[c * S : (c + 1) * S].reshape(-1))[0] for c in range(NCORES)
    ]
    nch = max(1, int(np.ceil(max(len(r) for r in rows_per_core) / P)))

    in_maps = []
    for c in range(NCORES):
        rows = rows_per_core[c]
        nslot = nch * P
        feat_c = feat[c * S : (c + 1) * S].reshape(S * A, D)
        feat_cmp = np.zeros((nslot, D), np.float16)
        feat_cmp[: len(rows)] = feat_c[rows].astype(np.float16)
        mbd = np.zeros((nch, P, S), np.float32)
        for i, r in enumerate(rows):
            mbd[i // P, i % P, r // A] = 1.0
        in_maps.append(
            {
                "feat": _pm(feat_cmp, nch),
                "mbdt": np.ascontiguousarray(mbd.transpose(1, 0, 2))
                .reshape(P, nch * S)
                .astype(np.float16),
                "w": w_pm,
                "bias": b,
                "ones": ones,
            }
        )
    return in_maps, nch


_NC_CACHE: dict = {}


def run(inputs: dict, trace: bool = False):
    from concourse.bass_utils import run_bass_kernel_spmd

    in_maps, nch = _host_prep(inputs)
    if nch not in _NC_CACHE:
        _NC_CACHE[nch] = build_nc(nch)
    nc = _NC_CACHE[nch]
    res = run_bass_kernel_spmd(nc, in_maps, list(range(NCORES)), trace=trace)
    out = np.concatenate([res.results[c]["out"] for c in range(NCORES)], axis=0)
    return out, res


def kernel(**inputs) -> np.ndarray:
    out, _ = run(inputs)
    return out


# revision 53
# speedup vs baseline: 1.1222x; 1.0276x over previous
"""Trainium2 Bass kernel for nn_Meta_Graph1_40114994545303 (gnn_message_passing).

Math: the reference returns only the global-node row of the GCN output.
With mask = (attribute_label > 0), star adjacency means
    out[s, :] = tanh( (sum_a mask[s,a] * attribute_feat[s,a,:]) @ W + b )
and x never reaches the output (adj[A, A] = 0). Data-parallel over batch,
32 samples per core on 8 cores; the kernel is HBM-bandwidth-bound, so:

- Dead-input elimination: rows with mask 0 have structurally-zero
  coefficients in the adjacency operand (same as x, which is never shipped),
  so the host stages only the live feat rows (~2.3MB vs 4MB), zero-padded to
  a 128-row-chunk multiple, partition-major so the DMA moves 16KB-contiguous
  runs per partition at line rate.
- Stage 1: masked sum as block-diagonal matmul (mask stationary, feat
  moving, four 512-col tiles packed in one PSUM bank via tile_position);
  DVE 32x32 block transposes (batched 4 blocks/instruction) build the
  stage-2 stationary, hidden under the W stream.
- Stage 2 chases the W stream k2-major; W replicated per core (collectives
  measure ~90us for even a 128KB AllGather on this runtime -- not viable).
- Bias folded in as a rank-1 matmul accumulated first (off the tail path);
  one full-width tanh on the scalar engine, then the output DMAs split
  across both HWDGE queues.
"""

import numpy as np

import concourse.bacc as bacc
import concourse.mybir as mybir

B, A, D = 256, 32, 2048
NCORES = 8
S = B // NCORES  # 32 samples per core
P = 128
KC2 = D // P  # 16 k-chunks in stage 2 (contraction over d_in)
NT = D // 512  # 4 psum-bank-wide column tiles
F32 = mybir.dt.float32
F16 = mybir.dt.float16

# W stream split across both HWDGE queues. Stage 2 consumes chunks in k2
# order, so the LAST bytes to arrive must be the LAST chunks in PE order:
# scalar carries the early chunks 0..5 (they land while sync still streams),
# sync carries 6..15 behind feat and finishes with chunk 15 — only its 4
# matmuls remain after the stream ends.
# single-chunk final groups: chunk 14's matmuls need not wait for chunk 15's
# DMA-completion receipt (~1.4us), halving the post-stream matmul tail
WCH = [4, 2, 4, 4, 1, 1]
WST = [0, 4, 6, 10, 14, 15]
NW = len(WCH)
W_SYNC_GROUPS = (2, 3, 4, 5)
W_SCALAR_GROUPS = (0, 1)


def build_nc(nch: int):
    cdt = F16
    nc = bacc.Bacc("TRN2", target_bir_lowering=False, debug=False)

    featd = nc.dram_tensor("feat", [P, nch * D], cdt, kind="ExternalInput")
    wd = nc.dram_tensor("w", [P, KC2 * D], cdt, kind="ExternalInput")
    mbdt = nc.dram_tensor("mbdt", [P, nch * S], cdt, kind="ExternalInput")
    bias = nc.dram_tensor("bias", [1, D], cdt, kind="ExternalInput")
    onesd = nc.dram_tensor("ones", [1, S], cdt, kind="ExternalInput")
    out = nc.dram_tensor("out", [S, D], F32, kind="ExternalOutput")

    from contextlib import ExitStack

    with ExitStack() as ctx:
        feat_sb = ctx.enter_context(nc.sbuf_tensor([P, nch, D], cdt))
        w_sb = ctx.enter_context(nc.sbuf_tensor([P, KC2, D], cdt))
        mbdt_sb = ctx.enter_context(nc.sbuf_tensor([P, nch, S], cdt))
        bias_sb = ctx.enter_context(nc.sbuf_tensor([1, D], cdt))
        ones_sb = ctx.enter_context(nc.sbuf_tensor([1, S], cdt))
        msc_sb = ctx.enter_context(nc.sbuf_tensor([P, 512], cdt))
        msT_sb = ctx.enter_context(nc.sbuf_tensor([P, KC2, S], cdt))
        out_sb = ctx.enter_context(nc.sbuf_tensor([P, 512], F32))
        pm_bank = ctx.enter_context(nc.psum_tensor([P, 512], F32))
        po_bank = ctx.enter_context(nc.psum_tensor([P, 512], F32))
        fsems = [ctx.enter_context(nc.semaphore(f"fs{g}")) for g in range(2)]
        wsems = [ctx.enter_context(nc.semaphore(f"ws{g}")) for g in range(NW)]
        csem = ctx.enter_context(nc.semaphore("csem"))
        s1_sem = ctx.enter_context(nc.semaphore("s1_sem"))
        tr_sem = ctx.enter_context(nc.semaphore("tr_sem"))
        s2_sem = ctx.enter_context(nc.semaphore("s2_sem"))
        act_sem = ctx.enter_context(nc.semaphore("act_sem"))
        osem = ctx.enter_context(nc.semaphore("osem"))
        block = ctx.enter_context(nc.Block(no_gpsimd_drain=True))

        # feat DMA split points (chunk counts per group)
        if nch == 1:
            FS = [(0, 1)]
        else:
            FS = [(0, (nch + 1) // 2), ((nch + 1) // 2, nch)]

        def w_dma(eng, g):
            st, ln = WST[g], WCH[g]
            eng.dma_start(
                w_sb[:, st : st + ln, :],
                wd[:, st * D : (st + ln) * D].rearrange("p (c d) -> p c d", d=D),
            ).then_inc(wsems[g], 16)

        @block.sync
        def _(sync):
            for g, (a0, a1) in enumerate(FS):
                sync.dma_start(
                    feat_sb[:, a0:a1, :],
                    featd[:, a0 * D : a1 * D].rearrange("p (c d) -> p c d", d=D),
                ).then_inc(fsems[g], 16)
            for g in W_SYNC_GROUPS:
                w_dma(sync, g)
            sync.wait_ge(act_sem, 1)
            # out DMA, left free-dim half: SBUF side stays 2D [128, 256]; the
            # DRAM side is a 3D AP (n, j, c) matching the (n j) packing
            sync.dma_start(
                out[:].rearrange("j (n c) -> n j c", c=512)[:, :, 0:256],
                out_sb[:, 0:256],
            ).then_inc(osem, 16)
            sync.wait_ge(osem, 16)

        @block.scalar
        def _(scalar):
            # tiny consts first on the otherwise-idle scalar queue, then the
            # tail of the W stream (arrives early, off the stage-2 pace)
            scalar.dma_start(
                mbdt_sb[:], mbdt[:].rearrange("p (k j) -> p k j", k=nch)
            ).then_inc(csem, 16)
            scalar.dma_start(bias_sb[:], bias[:]).then_inc(csem, 16)
            scalar.dma_start(ones_sb[:], onesd[:]).then_inc(csem, 16)
            for g in W_SCALAR_GROUPS:
                w_dma(scalar, g)
            # tanh in two free-dim halves (full 128 partitions each, so no
            # partition-offset activation); the left half's out DMA on sync
            # overlaps the right half's tanh
            scalar.wait_ge(s2_sem, NT)
            nc.scalar.activation(
                out_sb[:, 0:256],
                po_bank[:, 0:256],
                mybir.ActivationFunctionType.Tanh,
            ).then_inc(act_sem, 1)
            nc.scalar.activation(
                out_sb[:, 256:512],
                po_bank[:, 256:512],
                mybir.ActivationFunctionType.Tanh,
            ).then_inc(act_sem, 1)
            scalar.wait_ge(act_sem, 2)
            scalar.dma_start(
                out[:].rearrange("j (n c) -> n j c", c=512)[:, :, 256:512],
                out_sb[:, 256:512],
            ).then_inc(osem, 16)
            scalar.wait_ge(osem, 32)

        @block.vector
        def _(vector):
            # s (stage-1 psum) -> fp16, then 32x32 block transposes into the
            # stage-2 stationary; 4 strided blocks per DVE instruction
            vector.wait_ge(s1_sem, 1)
            nc.vector.tensor_copy(msc_sb[:], pm_bank[:])
            nc.vector.drain()
            # 32x32 block transposes, 4 strided blocks per DVE instruction:
            # blocks (n, q=rg+4t) share output partition rows rg*32 and map to
            # k2 = 4n+t
            lastt = None
            for n in range(NT):
                for rg in range(NT):
                    lastt = nc.vector.transpose(
                        msT_sb[rg * S : (rg + 1) * S, 4 * n : 4 * n + 4, :],
                        msc_sb[n * S : (n + 1) * S, :]
                        .rearrange("p (c q j) -> p c q j", q=NT, j=S)[
                            :, :, rg : rg + 1, :
                        ]
                        .rearrange("p c q j -> p (c q) j"),
                    )
            lastt.then_inc(tr_sem, 1)

        @block.tensor
        def _(tensor):
            tensor.wait_ge(csem, 48)  # mbdt/bias/ones resident
            # bias as the FIRST accumulation into po_bank (off the tail path)
            for n in range(NT):
                nc.tensor.matmul(
                    po_bank[n * S : (n + 1) * S, :],
                    ones_sb[:],
                    bias_sb[:, n * 512 : (n + 1) * 512],
                    start=True,
                    stop=False,
                    tile_position=(0, n * S),
                    skip_group_check=True,
                )
            # stage 1: s[j, d] = sum_slot mbd[slot, j] * feat[slot, d]
            # (mask stationary, feat moving; 4 column tiles packed into one
            # PSUM bank at partition offsets 0/32/64/96)
            last = None
            for k in range(nch):
                for g, (a0, _) in enumerate(FS):
                    if k == a0:
                        tensor.wait_ge(fsems[g], 16)
                for n in range(NT):
                    last = nc.tensor.matmul(
                        pm_bank[n * S : (n + 1) * S, :],
                        mbdt_sb[:, k, :],
                        feat_sb[:, k, n * 512 : (n + 1) * 512],
                        start=(k == 0),
                        stop=(k == nch - 1),
                        tile_position=(0, n * S),
                        skip_group_check=True,
                    )
            last.then_inc(s1_sem, 1)
            tensor.wait_ge(tr_sem, 1)
            # stage 2 k2-major so the PE chases the W stream; at the final
            # k-chunk each column tile signals s2 so tanh/output pipeline
            for g in range(NW):
                tensor.wait_ge(wsems[g], 16)
                for c in range(WCH[g]):
                    k2 = WST[g] + c
                    for n in range(NT):
                        mm = nc.tensor.matmul(
                            po_bank[n * S : (n + 1) * S, :],
                            msT_sb[:, k2, :],
                            w_sb[:, k2, n * 512 : (n + 1) * 512],
                            start=False,
                            stop=(k2 == KC2 - 1),
                            tile_position=(0, n * S),
                            skip_group_check=True,
                        )
                        if k2 == KC2 - 1:
                            mm.then_inc(s2_sem, 1)

    nc.compile()
    return nc


def _pm(x, nchunks):
    d = x.shape[1]
    return np.ascontiguousarray(
        x.reshape(nchunks, P, d).transpose(1, 0, 2).reshape(P, nchunks * d)
    )


def _host_prep(inputs: dict):
    feat = np.asarray(inputs["attribute_feat"], dtype=np.float32)
    label = np.asarray(inputs["attribute_label"])
    w = np.asarray(inputs["W"], dtype=np.float32).astype(np.float16)
    b = np.asarray(inputs["b"], dtype=np.float32).reshape(1, D).astype(np.float16)
    mask = label > 0

    w_pm = _pm(w, KC2)
    ones = np.ones((1, S), np.float16)

    rows_per_core = [
        np.nonzero(mask[c * S : (c + 1) * S].reshape(-1))[0] for c in range(NCORES)
    ]
    nch = max(1, int(np.ceil(max(len(r) for r in rows_per_core) / P)))

    in_maps = []
    for c in range(NCORES):
        rows = rows_per_core[c]
        nslot = nch * P
        feat_c = feat[c * S : (c + 1) * S].reshape(S * A, D)
        feat_cmp = np.zeros((nslot, D), np.float16)
        feat_cmp[: len(rows)] = feat_c[rows].astype(np.float16)
        mbd = np.zeros((nch, P, S), np.float32)
        for i, r in enumerate(rows):
            mbd[i // P, i % P, r // A] = 1.0
        in_maps.append(
            {
                "feat": _pm(feat_cmp, nch),
                "mbdt": np.ascontiguousarray(mbd.transpose(1, 0, 2))
                .reshape(P, nch * S)
                .astype(np.float16),
                "w": w_pm,
                "bias": b,
                "ones": ones,
            }
        )
    return in_maps, nch


_NC_CACHE: dict = {}


def run(inputs: dict, trace: bool = False):
    from concourse.bass_utils import run_bass_kernel_spmd

    in_maps, nch = _host_prep(inputs)
    if nch not in _NC_CACHE:
        _NC_CACHE[nch] = build_nc(nch)
    nc = _NC_CACHE[nch]
    res = run_bass_kernel_spmd(nc, in_maps, list(range(NCORES)), trace=trace)
    out = np.concatenate([res.results[c]["out"] for c in range(NCORES)], axis=0)
    return out, res


def kernel(**inputs) -> np.ndarray:
    out, _ = run(inputs)
    return out


# revision 61
# speedup vs baseline: 1.1414x; 1.0171x over previous
"""Trainium2 Bass kernel for nn_Meta_Graph1_40114994545303 (gnn_message_passing).

Math: the reference returns only the global-node row of the GCN output.
With mask = (attribute_label > 0), star adjacency means
    out[s, :] = tanh( (sum_a mask[s,a] * attribute_feat[s,a,:]) @ W + b )
and x never reaches the output (adj[A, A] = 0). Data-parallel over batch,
32 samples per core on 8 cores; the kernel is HBM-bandwidth-bound, so:

- Dead-input elimination: rows with mask 0 have structurally-zero
  coefficients in the adjacency operand (same as x, which is never shipped),
  so the host stages only the live feat rows (~2.3MB vs 4MB), zero-padded to
  a 128-row-chunk multiple, partition-major so the DMA moves 16KB-contiguous
  runs per partition at line rate.
- Stage 1: masked sum as block-diagonal matmul (mask stationary, feat
  moving, four 512-col tiles packed in one PSUM bank via tile_position);
  DVE 32x32 block transposes (batched 4 blocks/instruction) build the
  stage-2 stationary, hidden under the W stream.
- Stage 2 chases the W stream k2-major; W replicated per core (collectives
  measure ~90us for even a 128KB AllGather on this runtime -- not viable).
- Bias folded in as a rank-1 matmul accumulated first (off the tail path);
  one full-width tanh on the scalar engine, then the output DMAs split
  across both HWDGE queues.
"""

import numpy as np

import concourse.bacc as bacc
import concourse.mybir as mybir

B, A, D = 256, 32, 2048
NCORES = 8
S = B // NCORES  # 32 samples per core
P = 128
KC2 = D // P  # 16 k-chunks in stage 2 (contraction over d_in)
NT = D // 512  # 4 psum-bank-wide column tiles
F32 = mybir.dt.float32
F16 = mybir.dt.float16

# W stream split across both HWDGE queues. Stage 2 consumes chunks in k2
# order, so the LAST bytes to arrive must be the LAST chunks in PE order:
# scalar carries the early chunks 0..5 (they land while sync still streams),
# sync carries 6..15 behind feat and finishes with chunk 15 — only its 4
# matmuls remain after the stream ends.
# single-chunk final groups: chunk 14's matmuls need not wait for chunk 15's
# DMA-completion receipt (~1.4us), halving the post-stream matmul tail
WCH = [4, 2, 4, 4, 1, 1]
WST = [0, 4, 6, 10, 14, 15]
NW = len(WCH)
W_SYNC_GROUPS = (2, 3, 4, 5)
W_SCALAR_GROUPS = (0, 1)


def build_nc(nch: int, klast: int):
    """nch full 128-row chunks plus one partial chunk of klast (0/32/64/96)
    rows -- the compacted feat slot count is padded to 32 rows instead of 128,
    trimming dead zero bytes from the stream."""
    cdt = F16
    nc = bacc.Bacc("TRN2", target_bir_lowering=False, debug=False)

    featd = nc.dram_tensor("feat", [P, nch * D], cdt, kind="ExternalInput")
    wd = nc.dram_tensor("w", [P, KC2 * D], cdt, kind="ExternalInput")
    mbdt = nc.dram_tensor("mbdt", [P, nch * S], cdt, kind="ExternalInput")
    if klast:
        featpd = nc.dram_tensor("featp", [klast, D], cdt, kind="ExternalInput")
        mbdpd = nc.dram_tensor("mbdp", [klast, S], cdt, kind="ExternalInput")
    bias = nc.dram_tensor("bias", [1, D], cdt, kind="ExternalInput")
    onesd = nc.dram_tensor("ones", [1, S], cdt, kind="ExternalInput")
    out = nc.dram_tensor("out", [S, D], F32, kind="ExternalOutput")

    from contextlib import ExitStack

    with ExitStack() as ctx:
        feat_sb = ctx.enter_context(nc.sbuf_tensor([P, nch, D], cdt))
        w_sb = ctx.enter_context(nc.sbuf_tensor([P, KC2, D], cdt))
        mbdt_sb = ctx.enter_context(nc.sbuf_tensor([P, nch, S], cdt))
        if klast:
            featp_sb = ctx.enter_context(nc.sbuf_tensor([klast, D], cdt))
            mbdp_sb = ctx.enter_context(nc.sbuf_tensor([klast, S], cdt))
        bias_sb = ctx.enter_context(nc.sbuf_tensor([1, D], cdt))
        ones_sb = ctx.enter_context(nc.sbuf_tensor([1, S], cdt))
        msc_sb = ctx.enter_context(nc.sbuf_tensor([P, 512], cdt))
        msT_sb = ctx.enter_context(nc.sbuf_tensor([P, KC2, S], cdt))
        out_sb = ctx.enter_context(nc.sbuf_tensor([P, 512], F32))
        pm_bank = ctx.enter_context(nc.psum_tensor([P, 512], F32))
        po_bank = ctx.enter_context(nc.psum_tensor([P, 512], F32))
        fsems = [ctx.enter_context(nc.semaphore(f"fs{g}")) for g in range(2)]
        fpsem = ctx.enter_context(nc.semaphore("fpsem"))
        wsems = [ctx.enter_context(nc.semaphore(f"ws{g}")) for g in range(NW)]
        csem = ctx.enter_context(nc.semaphore("csem"))
        s1_sem = ctx.enter_context(nc.semaphore("s1_sem"))
        tr_sem = ctx.enter_context(nc.semaphore("tr_sem"))
        s2_sem = ctx.enter_context(nc.semaphore("s2_sem"))
        act_sem = ctx.enter_context(nc.semaphore("act_sem"))
        osem = ctx.enter_context(nc.semaphore("osem"))
        block = ctx.enter_context(nc.Block(no_gpsimd_drain=True))

        # feat DMA split points (chunk counts per group)
        if nch == 1:
            FS = [(0, 1)]
        else:
            FS = [(0, (nch + 1) // 2), ((nch + 1) // 2, nch)]

        def w_dma(eng, g):
            st, ln = WST[g], WCH[g]
            eng.dma_start(
                w_sb[:, st : st + ln, :],
                wd[:, st * D : (st + ln) * D].rearrange("p (c d) -> p c d", d=D),
            ).then_inc(wsems[g], 16)

        @block.sync
        def _(sync):
            for g, (a0, a1) in enumerate(FS):
                sync.dma_start(
                    feat_sb[:, a0:a1, :],
                    featd[:, a0 * D : a1 * D].rearrange("p (c d) -> p c d", d=D),
                ).then_inc(fsems[g], 16)
            for g in W_SYNC_GROUPS:
                w_dma(sync, g)
            sync.wait_ge(act_sem, 1)
            # out DMA, left free-dim half: SBUF side stays 2D [128, 256]; the
            # DRAM side is a 3D AP (n, j, c) matching the (n j) packing
            sync.dma_start(
                out[:].rearrange("j (n c) -> n j c", c=512)[:, :, 0:256],
                out_sb[:, 0:256],
            ).then_inc(osem, 16)
            sync.wait_ge(osem, 16)

        @block.scalar
        def _(scalar):
            # tiny consts first on the otherwise-idle scalar queue, then the
            # tail of the W stream (arrives early, off the stage-2 pace)
            scalar.dma_start(
                mbdt_sb[:], mbdt[:].rearrange("p (k j) -> p k j", k=nch)
            ).then_inc(csem, 16)
            scalar.dma_start(bias_sb[:], bias[:]).then_inc(csem, 16)
            scalar.dma_start(ones_sb[:], onesd[:]).then_inc(csem, 16)
            if klast:
                scalar.dma_start(mbdp_sb[:], mbdpd[:]).then_inc(csem, 16)
                scalar.dma_start(featp_sb[:], featpd[:]).then_inc(fpsem, 16)
            for g in W_SCALAR_GROUPS:
                w_dma(scalar, g)
            # tanh in two free-dim halves (full 128 partitions each, so no
            # partition-offset activation); the left half's out DMA on sync
            # overlaps the right half's tanh
            scalar.wait_ge(s2_sem, NT)
            nc.scalar.activation(
                out_sb[:, 0:256],
                po_bank[:, 0:256],
                mybir.ActivationFunctionType.Tanh,
            ).then_inc(act_sem, 1)
            nc.scalar.activation(
                out_sb[:, 256:512],
                po_bank[:, 256:512],
                mybir.ActivationFunctionType.Tanh,
            ).then_inc(act_sem, 1)
            scalar.wait_ge(act_sem, 2)
            scalar.dma_start(
                out[:].rearrange("j (n c) -> n j c", c=512)[:, :, 256:512],
                out_sb[:, 256:512],
            ).then_inc(osem, 16)
            scalar.wait_ge(osem, 32)

        @block.vector
        def _(vector):
            # s (stage-1 psum) -> fp16, then 32x32 block transposes into the
            # stage-2 stationary; 4 strided blocks per DVE instruction
            vector.wait_ge(s1_sem, 1)
            nc.vector.tensor_copy(msc_sb[:], pm_bank[:])
            nc.vector.drain()
            # 32x32 block transposes, 4 strided blocks per DVE instruction:
            # blocks (n, q=rg+4t) share output partition rows rg*32 and map to
            # k2 = 4n+t
            lastt = None
            for n in range(NT):
                for rg in range(NT):
                    lastt = nc.vector.transpose(
                        msT_sb[rg * S : (rg + 1) * S, 4 * n : 4 * n + 4, :],
                        msc_sb[n * S : (n + 1) * S, :]
                        .rearrange("p (c q j) -> p c q j", q=NT, j=S)[
                            :, :, rg : rg + 1, :
                        ]
                        .rearrange("p c q j -> p (c q) j"),
                    )
            lastt.then_inc(tr_sem, 1)

        @block.tensor
        def _(tensor):
            tensor.wait_ge(csem, 64 if klast else 48)  # consts resident
            # bias as the FIRST accumulation into po_bank (off the tail path)
            for n in range(NT):
                nc.tensor.matmul(
                    po_bank[n * S : (n + 1) * S, :],
                    ones_sb[:],
                    bias_sb[:, n * 512 : (n + 1) * 512],
                    start=True,
                    stop=False,
                    tile_position=(0, n * S),
                    skip_group_check=True,
                )
            # stage 1: s[j, d] = sum_slot mbd[slot, j] * feat[slot, d]
            # (mask stationary, feat moving; 4 column tiles packed into one
            # PSUM bank at partition offsets 0/32/64/96)
            last = None
            for k in range(nch):
                for g, (a0, _) in enumerate(FS):
                    if k == a0:
                        tensor.wait_ge(fsems[g], 16)
                for n in range(NT):
                    last = nc.tensor.matmul(
                        pm_bank[n * S : (n + 1) * S, :],
                        mbdt_sb[:, k, :],
                        feat_sb[:, k, n * 512 : (n + 1) * 512],
                        start=(k == 0),
                        stop=(k == nch - 1 and not klast),
                        tile_position=(0, n * S),
                        skip_group_check=True,
                    )
            if klast:
                tensor.wait_ge(fpsem, 16)
                for n in range(NT):
                    last = nc.tensor.matmul(
                        pm_bank[n * S : (n + 1) * S, :],
                        mbdp_sb[:],
                        featp_sb[:, n * 512 : (n + 1) * 512],
                        start=(nch == 0),
                        stop=True,
                        tile_position=(0, n * S),
                        skip_group_check=True,
                    )
            last.then_inc(s1_sem, 1)
            tensor.wait_ge(tr_sem, 1)
            # stage 2 k2-major so the PE chases the W stream; at the final
            # k-chunk each column tile signals s2 so tanh/output pipeline
            for g in range(NW):
                tensor.wait_ge(wsems[g], 16)
                for c in range(WCH[g]):
                    k2 = WST[g] + c
                    for n in range(NT):
                        mm = nc.tensor.matmul(
                            po_bank[n * S : (n + 1) * S, :],
                            msT_sb[:, k2, :],
                            w_sb[:, k2, n * 512 : (n + 1) * 512],
                            start=False,
                            stop=(k2 == KC2 - 1),
                            tile_position=(0, n * S),
                            skip_group_check=True,
                        )
                        if k2 == KC2 - 1:
                            mm.then_inc(s2_sem, 1)

    nc.compile()
    return nc


def _pm(x, nchunks):
    d = x.shape[1]
    return np.ascontiguousarray(
        x.reshape(nchunks, P, d).transpose(1, 0, 2).reshape(P, nchunks * d)
    )


def _host_prep(inputs: dict):
    feat = np.asarray(inputs["attribute_feat"], dtype=np.float32)
    label = np.asarray(inputs["attribute_label"])
    w = np.asarray(inputs["W"], dtype=np.float32).astype(np.float16)
    b = np.asarray(inputs["b"], dtype=np.float32).reshape(1, D).astype(np.float16)
    mask = label > 0

    w_pm = _pm(w, KC2)
    ones = np.ones((1, S), np.float16)

    rows_per_core = [
        np.nonzero(mask[c * S : (c + 1) * S].reshape(-1))[0] for c in range(NCORES)
    ]
    max_n = max(len(r) for r in rows_per_core)
    nch = max_n // P
    klast = -(-max(max_n - nch * P, 0) // 32) * 32  # round up to 32
    if klast == P or nch == 0:
        # fold a full-size partial back into a full chunk; keep nch >= 1
        nch += 1
        klast = 0

    in_maps = []
    for c in range(NCORES):
        rows = rows_per_core[c]
        nslot = nch * P + klast
        feat_c = feat[c * S : (c + 1) * S].reshape(S * A, D)
        feat_cmp = np.zeros((nslot, D), np.float16)
        feat_cmp[: len(rows)] = feat_c[rows].astype(np.float16)
        mbd = np.zeros((nch, P, S), np.float32)
        mbdp = np.zeros((klast, S), np.float32)
        for i, r in enumerate(rows):
            if i < nch * P:
                mbd[i // P, i % P, r // A] = 1.0
            else:
                mbdp[i - nch * P, r // A] = 1.0
        m = {
            "feat": _pm(feat_cmp[: nch * P], nch),
            "mbdt": np.ascontiguousarray(mbd.transpose(1, 0, 2))
            .reshape(P, nch * S)
            .astype(np.float16),
            "w": w_pm,
            "bias": b,
            "ones": ones,
        }
        if klast:
            m["featp"] = np.ascontiguousarray(feat_cmp[nch * P :])
            m["mbdp"] = mbdp.astype(np.float16)
        in_maps.append(m)
    return in_maps, nch, klast


_NC_CACHE: dict = {}


def run(inputs: dict, trace: bool = False):
    from concourse.bass_utils import run_bass_kernel_spmd

    in_maps, nch, klast = _host_prep(inputs)
    key = (nch, klast)
    if key not in _NC_CACHE:
        _NC_CACHE[key] = build_nc(nch, klast)
    nc = _NC_CACHE[key]
    res = run_bass_kernel_spmd(nc, in_maps, list(range(NCORES)), trace=trace)
    out = np.concatenate([res.results[c]["out"] for c in range(NCORES)], axis=0)
    return out, res


def kernel(**inputs) -> np.ndarray:
    out, _ = run(inputs)
    return out
